# revision 41
# baseline (speedup 1.0000x reference)
"""BertSelfAttention (with value-bypass relu-add) on 8 Trainium2 NeuronCores.

Strategy: data-parallel over batch B=8 -> one batch element per core, no
collectives. Per core, attention is computed in a transposed-softmax layout:

  qT, kT = (x @ W.T).T + b + r.T      [H, L] (heads are 64-row slices)
  v      = x @ Wv.T + r               [Lk, H], augmented with a ones column
  S.T    = kT_head.T-matmul           [lk, lq]  (keys on partitions)
  E      = exp(S.T * 1/8 + maskbias)  (mask folded into the activation bias;
                                       exp(-1e9) == 0 kills masked keys)
  PV     = [v_head | 1].T @ E         -> rows 0..63 unnormalized attn.T,
                                         row 64 = softmax denominator (free)
  attnT  = PV[0:64] * bcast(1/PV[64]) (approx-recip + gpsimd partition bcast)
  out    = attnT.T-matmul with Wo.T + bo

Masked keys are compacted away on the host (gather unmasked key rows, pad to
a multiple of 128; padded keys get x=0 and a -1e9 bias so exp()==0 exactly).

The relu bypass r = 0.5*relu(x) is computed ON-CORE from the already-loaded
x tiles (relu(0.5x) == 0.5relu(x)) as a DVE (vector) op, so no rqT/rkT DMA.
Per-partition q/k biases are folded into the psum+r add via
scalar_tensor_tensor.

Input DMAs are split per k-slab / per head-pair weight block and spread
across the sync/scalar HW DGE queues plus the gpsimd SW queue, ordered so
the k-projection's operands land first and later pairs' weights stream
just-in-time from inside the pipeline; Wo loads in background (needed only
at the out-projection); output DMAs alternate between the two HW queues.

Pipeline order: qk0 st0 qk1 st1 v* | pv0 qk2 st2 | pv1 qk3 st3 | pv2 qk4
st4 | qk5 st5 pv3 pv4 pv5 | out-proj — scores/exp run 1-2 pairs ahead of PV
so the scalar engine's exp stream and the PV normalize chains hide under PE
matmuls; the Tile scheduler additionally prefills out-projection
accumulation (k<=4) into the late-PV bubbles.
"""

import os
import sys

for _p in ("/opt/trn_rl_repo", "/root/.axon_site/_ro/trn_rl_repo"):
    if os.path.isdir(_p) and _p not in sys.path:
        sys.path.insert(0, _p)

import ml_dtypes
import numpy as np

import concourse.bacc as bacc
import concourse.bass as bass
import concourse.mybir as mybir
import concourse.tile as tile
from concourse.bass_utils import run_bass_kernel_spmd

B, L, H = 8, 1024, 768
NH, DH = 12, 64
SCALE = 1.0 / 8.0
NEG = -1e9
KT = H // 128            # 6 contraction tiles over hidden dim
LQT = L // 128           # 8 query row-tiles
F32 = mybir.dt.float32
BF16 = mybir.dt.bfloat16

LAST_EXEC_NS = None
LAST_RESULTS = None
_CACHE = {}


def _chunks(total, maxc):
    """Split `total` into nearly-equal chunks of at most `maxc`, multiples of 64."""
    n = -(-total // maxc)
    base = total // n
    base -= base % 64
    sizes = [base] * n
    sizes[-1] = total - base * (n - 1)
    out, off = [], 0
    for s in sizes:
        out.append((off, s))
        off += s
    return out


def _build(lk, nmax, has_bo):
    """Build + compile the 8-core SPMD program; lk = padded key count
    (tile allocation), nmax = max real key count (compute bound)."""
    lkt = lk // 128          # key row-tiles
    rows_of = [min(128, nmax - 128 * i) for i in range(lkt)]
    nc = bacc.Bacc("TRN2", target_bir_lowering=False, debug=False, num_devices=B)

    xT = nc.dram_tensor("xT", [128, KT, L], BF16, kind="ExternalInput")
    xTk = nc.dram_tensor("xTk", [128, KT, lk], BF16, kind="ExternalInput")
    rv = nc.dram_tensor("rv", [lk, H], BF16, kind="ExternalInput")
    wq = nc.dram_tensor("wqT", [128, KT, H], BF16, kind="ExternalInput")
    wk = nc.dram_tensor("wkT", [128, KT, H], BF16, kind="ExternalInput")
    wv = nc.dram_tensor("wvT", [128, KT, H], BF16, kind="ExternalInput")
    wo = nc.dram_tensor("woT", [128, KT, H], BF16, kind="ExternalInput")
    mb = nc.dram_tensor("maskb", [128, lkt], F32, kind="ExternalInput")
    bqp = nc.dram_tensor("bqp", [128, KT], F32, kind="ExternalInput")
    bkp = nc.dram_tensor("bkp", [128, KT], F32, kind="ExternalInput")
    out_d = nc.dram_tensor("out", [L, H], F32, kind="ExternalOutput")
    bo_d = nc.dram_tensor("bo", [H], F32, kind="ExternalInput") if has_bo else None

    kchunks = _chunks(nmax, 512)     # kT free-dim chunks (N per matmul)
    exp_t = mybir.ActivationFunctionType.Exp
    op_add = mybir.AluOpType.add
    op_mult = mybir.AluOpType.mult
    op_max = mybir.AluOpType.max

    with tile.TileContext(nc) as tc:
        with (
            tc.tile_pool(name="persist", bufs=1) as persist,
            tc.tile_pool(name="xtp", bufs=1) as xtp,
            tc.tile_pool(name="wpool", bufs=1) as wpool,
            tc.tile_pool(name="rp", bufs=2) as rp,
            tc.tile_pool(name="ep", bufs=3) as ep,
            tc.tile_pool(name="rcp", bufs=3) as rcp,
            tc.tile_pool(name="bcp", bufs=3) as bcp,
            tc.tile_pool(name="outp", bufs=3) as outp,
            tc.tile_pool(name="psum", bufs=1, space="PSUM") as psum,
        ):
            mbt = persist.tile([128, lkt], F32, tag="mbt", name="mbt")
            qTt = [persist.tile([128, L], BF16, tag=f"qT{i}", name=f"qT{i}")
                   for i in range(KT)]
            kTt = [persist.tile([128, lk], BF16, tag=f"kT{i}", name=f"kT{i}")
                   for i in range(KT)]
            vaug = [persist.tile([128, NH, DH + 1], BF16, tag=f"va{i}", name=f"va{i}")
                    for i in range(lkt)]
            attnT = [persist.tile([128, L], BF16, tag=f"aT{i}", name=f"aT{i}")
                     for i in range(KT)]
            ones_s = persist.tile([128, NH], F32, tag="ones", name="ones")
            nc.vector.memset(ones_s[:], 1.0)
            zeros_t = persist.tile([128, L], BF16, tag="zeros", name="zeros")
            nc.vector.memset(zeros_t[:], 0.0)
            bq_sb = persist.tile([128, KT], F32, tag="bq", name="bq_sb")
            bk_sb = persist.tile([128, KT], F32, tag="bk", name="bk_sb")
            woa = persist.tile([128, KT, H], BF16, tag="woa", name="woa")
            woTt = [woa[:, k, :] for k in range(KT)]

            xTa = xtp.tile([128, KT, L], BF16, tag="xTa", name="xTa")
            xKa = xtp.tile([128, KT, lk], BF16, tag="xKa", name="xKa")
            # q/k weights as per-head-pair column blocks so pair p's
            # projections only wait on their own 196KB, not the full WqT/WkT
            wqp = [wpool.tile([128, KT, 128], BF16, tag=f"wqp{p}",
                              name=f"wqp{p}") for p in range(KT)]
            wkp = [wpool.tile([128, KT, 128], BF16, tag=f"wkp{p}",
                              name=f"wkp{p}") for p in range(KT)]
            wva = wpool.tile([128, KT, H], BF16, tag="wva", name="wva")
            xTt = [xTa[:, k, :] for k in range(KT)]
            xKt = [xKa[:, k, :] for k in range(KT)]
            wvt = [wva[:, k, :] for k in range(KT)]

            # --- input DMAs: spread over three queues, in first-use order.
            # sync: compacted keys (k-projection moving operand) + rv;
            # scalar: per-pair q/k weight blocks; gpsimd: xT + v weights +
            # small/late tensors.
            for k in range(KT):
                nc.sync.dma_start(xKa[:, k, :], xTk[:, k, :])
            # only the first two pairs' q/k weight blocks load upfront;
            # later pairs' blocks are issued from emit_qk(p-2) below so
            # they don't displace the front-critical x bytes.
            for p in range(2):
                nc.scalar.dma_start(wkp[p][:], wk[:, :, p * 128:(p + 1) * 128])
                nc.scalar.dma_start(wqp[p][:], wq[:, :, p * 128:(p + 1) * 128])
            # small tensors ride the scalar queue behind the first weight
            # blocks (all needed only by ~15us: exp bias, psum-add biases)
            nc.scalar.dma_start(mbt[:], mb[:])
            nc.scalar.dma_start(bk_sb[:], bkp[:])
            nc.scalar.dma_start(bq_sb[:], bqp[:])
            for k in range(KT):
                nc.gpsimd.dma_start(xTa[:, k, :], xT[:, k, :])
            # v weights ride the sync queue, which is idle once the
            # compacted keys have landed; behind xTa they'd arrive ~4us
            # too late for the v projection.
            for k in range(KT):
                nc.sync.dma_start(wva[:, k, :], wv[:, k, :])
            if has_bo:
                bo_bc = persist.tile([128, H], F32, tag="bo", name="bo_bc")
                bo_ap = bo_d.ap()
                nc.gpsimd.dma_start(
                    out=bo_bc[:],
                    in_=bass.AP(tensor=bo_ap.tensor, offset=0, ap=[[0, 128], [1, H]]),
                )

            # ---- v projection, natural layout [lk, H], augmented tiles ----
            def emit_v(lt):
                rows = rows_of[lt]
                rv_t = rp.tile([128, H], BF16, tag="rv", name="rv_t")
                nc.gpsimd.dma_start(rv_t[0:rows, :],
                                    rv[lt * 128:lt * 128 + rows, :])
                for ch in range(2):
                    ps = psum.tile([128, 512], F32, tag="ps", bufs=2, name="psv")
                    for k in range(KT):
                        nc.tensor.matmul(
                            ps[0:rows, 0:384],
                            xKt[k][:, lt * 128:lt * 128 + rows],
                            wvt[k][:, ch * 384:(ch + 1) * 384],
                            start=(k == 0), stop=(k == KT - 1),
                        )
                    nc.vector.tensor_add(
                        vaug[lt][0:rows, ch * 6:(ch + 1) * 6, 0:DH],
                        ps[0:rows, 0:384].rearrange("p (h d) -> p h d", d=DH),
                        rv_t[0:rows, ch * 384:(ch + 1) * 384].rearrange(
                            "p (h d) -> p h d", d=DH),
                    )
                nc.vector.tensor_copy(vaug[lt][0:rows, :, DH], ones_s[0:rows, :])

            def emit_qk(p, ramp=False):
                """q/k projections for head-pair p (= ho-tile p of each).

                ramp=True (first pairs, before any scores exist): allocate
                the q-side psum chunks from the idle "st" banks so the q
                and k projections accumulate concurrently while their
                input slabs trickle in from HBM, instead of q waiting for
                the k chunks to release the "ps" buffers."""
                if p + 2 < KT:
                    pn = p + 2
                    nc.scalar.dma_start(
                        wkp[pn][:], wk[:, :, pn * 128:(pn + 1) * 128])
                    nc.scalar.dma_start(
                        wqp[pn][:], wq[:, :, pn * 128:(pn + 1) * 128])
                for wt, b_sb, dst, rhs, xsrc, ck in (
                    (wkp, bk_sb, kTt, xKt, xKa, kchunks),
                    (wqp, bq_sb, qTt, xTt, xTa, ((0, 512), (512, 512))),
                ):
                    is_q = dst is qTt
                    nfree = ck[-1][0] + ck[-1][1]
                    r_t = rp.tile([128, L], BF16, tag="r", name="r_t")
                    # r = 0.5*relu(x) on-core: (x * 0.5) max 0
                    nc.vector.scalar_tensor_tensor(
                        r_t[:, 0:nfree], xsrc[:, p, 0:nfree], 0.5,
                        zeros_t[:, 0:nfree], op_mult, op_max)
                    for (o0, on) in ck:
                        if ramp and is_q:
                            pst = psum.tile([128, L], F32, tag="st", bufs=2,
                                            name="st_ps")
                            ps = pst[:, 0:512]
                        else:
                            ps = psum.tile([128, 512], F32, tag="ps", bufs=2,
                                           name="psq")
                        for k in range(KT):
                            nc.tensor.matmul(
                                ps[:, 0:on],
                                wt[p][:, k, :],
                                rhs[k][:, o0:o0 + on],
                                start=(k == 0), stop=(k == KT - 1),
                            )
                        # dst = (psum + bias_per_partition) + r
                        nc.vector.scalar_tensor_tensor(
                            dst[p][:, o0:o0 + on], ps[:, 0:on],
                            b_sb[:, p:p + 1], r_t[:, o0:o0 + on],
                            op_add, op_add)

            def emit_st(p):
                """Scores + exp for head pair p; returns exp tiles.

                hh-major so the exp tiles drain in the same order emit_pv
                consumes them (head hh's PV needs ex[hh, all i]); this lets
                PV start after 5 exps instead of 9."""
                ex = {}
                for hh, off in ((0, 0), (1, 64)):
                    for i in range(lkt):
                        rows = rows_of[i]
                        pss = psum.tile([128, L], F32, tag="st", bufs=2,
                                        name="st_ps")
                        for j in range(2):
                            nc.tensor.matmul(
                                pss[0:rows, j * 512:(j + 1) * 512],
                                kTt[p][off:off + DH, i * 128:i * 128 + rows],
                                qTt[p][off:off + DH, j * 512:(j + 1) * 512],
                                start=True, stop=True,
                            )
                        e = ep.tile([128, L], BF16, tag=f"ex{hh}_{i}",
                                    name=f"ex{hh}_{i}")
                        nc.scalar.activation(
                            e[0:rows, :], pss[0:rows, :], exp_t,
                            bias=mbt[0:rows, i:i + 1], scale=SCALE)
                        ex[hh, i] = e
                return ex

            def emit_pv(p, ex, tail=False):
                """PV + normalization for head pair p -> attnT.

                tail=True (last pairs, once the exp stream has drained and
                the scalar engine is idle): evacuate the PV PSUM tile to
                SBUF with a scalar copy so the bank frees at matmul rate
                instead of being held through the whole normalize chain."""
                for hh, off in ((0, 0), (1, 64)):
                    head = 2 * p + hh
                    for j in range(2):
                        pv = psum.tile([DH + 1, 512], F32, tag="pv",
                                       bufs=2, name="pv_ps")
                        for i in range(lkt):
                            rows = rows_of[i]
                            nc.tensor.matmul(
                                pv[:],
                                vaug[i][0:rows, head, :],
                                ex[hh, i][0:rows, j * 512:(j + 1) * 512],
                                start=(i == 0), stop=(i == lkt - 1),
                            )
                        if tail:
                            src = rcp.tile([DH + 1, 512], F32, tag="pvs",
                                           name="pvs_t")
                            nc.scalar.copy(src[:], pv[:])
                        else:
                            src = pv
                        # normalize: denom row -> partition-0 SBUF tile (the
                        # custom-DVE reciprocal needs a partition-0-based
                        # SBUF input), reciprocal, broadcast, multiply.
                        dn = rcp.tile([1, 512], F32, tag="dn", name="dn_t")
                        nc.vector.tensor_copy(dn[:], src[DH:DH + 1, :])
                        rc = rcp.tile([1, 512], F32, tag="rc", name="rc_t")
                        nc.vector.reciprocal_approx_fast(out=rc[:], in_=dn[:])
                        bc = bcp.tile([DH, 512], F32, tag="bc", name="bc_t")
                        nc.gpsimd.partition_broadcast(bc[:], rc[:])
                        nc.vector.tensor_mul(
                            attnT[p][off:off + DH, j * 512:(j + 1) * 512],
                            src[0:DH, :], bc[:])

            # software pipeline: scores/exp run one-to-two pairs ahead of
            # PV so the scalar engine's exp stream hides under PE matmuls;
            # st5 is pulled before pv3/pv4 so exp(5) has PE work to hide
            # under at the tail.  The v projection runs after st1 so its
            # input DMAs don't compete with the q/k path for HBM bandwidth
            # during the ramp.
            emit_qk(0, ramp=True)
            exs = {0: emit_st(0)}
            emit_qk(1, ramp=True)
            exs[1] = emit_st(1)
            for lt in range(lkt):
                emit_v(lt)
            # Wo load: issued here (gpsimd reaches it early) but only
            # needed by the out-projection, so it streams in background.
            nc.gpsimd.dma_start(woa[:], wo[:])
            emit_pv(0, exs.pop(0))
            emit_qk(2)
            exs[2] = emit_st(2)
            emit_pv(1, exs.pop(1))
            emit_qk(3)
            exs[3] = emit_st(3)
            emit_pv(2, exs.pop(2))
            emit_qk(4)
            exs[4] = emit_st(4)
            emit_qk(5)
            exs[5] = emit_st(5)
            emit_pv(3, exs.pop(3))
            emit_pv(4, exs.pop(4))
            emit_pv(5, exs.pop(5))

            # ---------------- output projection ----------------
            # the first two row-tiles' accumulators live on the "st" banks
            # (idle once the exp stream drains) so four chunks can prefill
            # their k<=4 partial sums under the last PV pairs instead of two
            for lt in range(LQT):
                so = outp.tile([128, H], F32, tag="so", name="so_t")
                for (o0, on) in ((0, 512), (512, 256)):
                    if lt < 2:
                        pst = psum.tile([128, L], F32, tag="st", bufs=2,
                                        name="st_ps")
                        ps = pst[:, 0:512]
                    else:
                        ps = psum.tile([128, 512], F32, tag="ps", bufs=2,
                                       name="pc")
                    for k in range(KT):
                        nc.tensor.matmul(
                            ps[:, 0:on],
                            attnT[k][:, lt * 128:(lt + 1) * 128],
                            woTt[k][:, o0:o0 + on],
                            start=(k == 0), stop=(k == KT - 1),
                        )
                    if has_bo:
                        nc.vector.tensor_add(
                            so[:, o0:o0 + on], ps[:, 0:on], bo_bc[:, o0:o0 + on])
                    else:
                        nc.vector.tensor_copy(so[:, o0:o0 + on], ps[:, 0:on])
                # alternate output DMAs over the two HW queues so the
                # final drain is ~2x faster
                eng = nc.sync if lt % 2 == 0 else nc.scalar
                eng.dma_start(
                    out_d[lt * 128:(lt + 1) * 128, :], so[:])

    nc.compile()
    return nc


def kernel(hidden_states, attention_mask, Wq, bq, Wk, bk, Wv, bv, Wo, bo):
    global LAST_EXEC_NS, LAST_RESULTS
    x = np.ascontiguousarray(np.asarray(hidden_states, dtype=np.float32))
    mask = np.asarray(attention_mask).astype(bool).reshape(B, L)
    bq = np.asarray(bq, dtype=np.float32)
    bk = np.asarray(bk, dtype=np.float32)
    bv = np.asarray(bv, dtype=np.float32)
    bo = np.asarray(bo, dtype=np.float32)
    has_bo = bool(np.any(bo))

    keep = [np.nonzero(~mask[b])[0] for b in range(B)]
    n_max = max(max(len(k) for k in keep), 64)
    lk = max(128, -(-n_max // 128) * 128)   # padded key count, multiple of 128

    key = (lk, n_max, has_bo)
    if key not in _CACHE:
        _CACHE[key] = _build(lk, n_max, has_bo)
    nc = _CACHE[key]

    bf = ml_dtypes.bfloat16

    def pk(a):
        """[H, X] -> [128, KT, X] (row-tile packing)."""
        return np.ascontiguousarray(
            a.reshape(KT, 128, a.shape[1]).swapaxes(0, 1))

    def pb(b_):
        """[H] -> [128, KT] per-slab bias packing."""
        return np.ascontiguousarray(b_.reshape(KT, 128).T)

    wqT = pk(np.asarray(Wq, dtype=np.float32).T.astype(bf))
    wkT = pk(np.asarray(Wk, dtype=np.float32).T.astype(bf))
    wvT = pk(np.asarray(Wv, dtype=np.float32).T.astype(bf))
    woT = pk(np.asarray(Wo, dtype=np.float32).T.astype(bf))
    bqpk = pb(bq)
    bkpk = pb(bk)

    in_maps = []
    for b in range(B):
        xb = x[b]                               # [L, H]
        rb = 0.5 * np.maximum(xb, 0.0)          # [L, H]
        idx = keep[b]
        n = len(idx)
        xk = np.zeros((lk, H), np.float32)      # compacted+padded key rows
        xk[:n] = xb[idx]
        rvb = np.zeros((lk, H), np.float32)
        rvb[:n] = rb[idx] + bv[None, :]
        mbias = np.full((lk,), NEG, np.float32)
        mbias[:n] = 0.0
        in_maps.append({
            "xT": pk(xb.T.astype(bf)),
            "xTk": pk(xk.T.astype(bf)),
            "rv": rvb.astype(bf),
            "wqT": wqT, "wkT": wkT, "wvT": wvT, "woT": woT,
            "bqp": bqpk, "bkp": bkpk,
            "maskb": np.ascontiguousarray(mbias.reshape(lk // 128, 128).T),
            **({"bo": bo} if has_bo else {}),
        })

    trace = bool(os.environ.get("BASS_KERNEL_TRACE"))
    res = run_bass_kernel_spmd(nc, in_maps, list(range(B)), trace=trace)
    LAST_EXEC_NS = res.exec_time_ns
    LAST_RESULTS = res
    return np.stack([res.results[b]["out"] for b in range(B)], axis=0)


# revision 42
# speedup vs baseline: 1.0199x; 1.0199x over previous
"""BertSelfAttention (with value-bypass relu-add) on 8 Trainium2 NeuronCores.

Strategy: data-parallel over batch B=8 -> one batch element per core, no
collectives. Per core, attention is computed in a transposed-softmax layout:

  qT, kT = (x @ W.T).T + b + r.T      [H, L] (heads are 64-row slices)
  v      = x @ Wv.T + r               [Lk, H], augmented with a ones column
  S.T    = kT_head.T-matmul           [lk, lq]  (keys on partitions)
  E      = exp(S.T * 1/8 + maskbias)  (mask folded into the activation bias;
                                       exp(-1e9) == 0 kills masked keys)
  PV     = [v_head | 1].T @ E         -> rows 0..63 unnormalized attn.T,
                                         row 64 = softmax denominator (free)
  attnT  = PV[0:64] * bcast(1/PV[64]) (approx-recip + gpsimd partition bcast)
  out    = attnT.T-matmul with Wo.T + bo

Masked keys are compacted away on the host (gather unmasked key rows, pad to
a multiple of 128; padded keys get x=0 and a -1e9 bias so exp()==0 exactly).

The relu bypass r = 0.5*relu(x) is computed ON-CORE from the already-loaded
x tiles (relu(0.5x) == 0.5relu(x)) as a DVE (vector) op, so no rqT/rkT DMA.
Per-partition q/k biases are folded into the psum+r add via
scalar_tensor_tensor.

Input DMAs are split per k-slab / per head-pair weight block and spread
across the sync/scalar HW DGE queues plus the gpsimd SW queue, ordered so
the k-projection's operands land first and later pairs' weights stream
just-in-time from inside the pipeline; Wo loads in background (needed only
at the out-projection); output DMAs alternate between the two HW queues.

Pipeline order: qk0 st0 qk1 st1 v* | pv0 qk2 st2 | pv1 qk3 st3 | pv2 qk4
st4 | qk5 st5 pv3 pv4 pv5 | out-proj — scores/exp run 1-2 pairs ahead of PV
so the scalar engine's exp stream and the PV normalize chains hide under PE
matmuls; the Tile scheduler additionally prefills out-projection
accumulation (k<=4) into the late-PV bubbles.
"""

import os
import sys

for _p in ("/opt/trn_rl_repo", "/root/.axon_site/_ro/trn_rl_repo"):
    if os.path.isdir(_p) and _p not in sys.path:
        sys.path.insert(0, _p)

import ml_dtypes
import numpy as np

import concourse.bacc as bacc
import concourse.bass as bass
import concourse.mybir as mybir
import concourse.tile as tile
from concourse.bass_utils import run_bass_kernel_spmd

B, L, H = 8, 1024, 768
NH, DH = 12, 64
SCALE = 1.0 / 8.0
NEG = -1e9
KT = H // 128            # 6 contraction tiles over hidden dim
LQT = L // 128           # 8 query row-tiles
F32 = mybir.dt.float32
BF16 = mybir.dt.bfloat16

LAST_EXEC_NS = None
LAST_RESULTS = None
_CACHE = {}


def _chunks(total, maxc):
    """Split `total` into nearly-equal chunks of at most `maxc`, multiples of 64."""
    n = -(-total // maxc)
    base = total // n
    base -= base % 64
    sizes = [base] * n
    sizes[-1] = total - base * (n - 1)
    out, off = [], 0
    for s in sizes:
        out.append((off, s))
        off += s
    return out


def _build(lk, nmax, has_bo):
    """Build + compile the 8-core SPMD program; lk = padded key count
    (tile allocation), nmax = max real key count (compute bound)."""
    lkt = lk // 128          # key row-tiles
    rows_of = [min(128, nmax - 128 * i) for i in range(lkt)]
    nc = bacc.Bacc("TRN2", target_bir_lowering=False, debug=False, num_devices=B)

    xT = nc.dram_tensor("xT", [128, KT, L], BF16, kind="ExternalInput")
    xTk = nc.dram_tensor("xTk", [128, KT, lk], BF16, kind="ExternalInput")
    rv = nc.dram_tensor("rv", [lk, H], BF16, kind="ExternalInput")
    wq = nc.dram_tensor("wqT", [128, KT, H], BF16, kind="ExternalInput")
    wk = nc.dram_tensor("wkT", [128, KT, H], BF16, kind="ExternalInput")
    wv = nc.dram_tensor("wvT", [128, KT, H], BF16, kind="ExternalInput")
    wo = nc.dram_tensor("woT", [128, KT, H], BF16, kind="ExternalInput")
    mb = nc.dram_tensor("maskb", [128, lkt], F32, kind="ExternalInput")
    bqp = nc.dram_tensor("bqp", [128, KT], F32, kind="ExternalInput")
    bkp = nc.dram_tensor("bkp", [128, KT], F32, kind="ExternalInput")
    out_d = nc.dram_tensor("out", [L, H], F32, kind="ExternalOutput")
    bo_d = nc.dram_tensor("bo", [H], F32, kind="ExternalInput") if has_bo else None

    kchunks = _chunks(nmax, 512)     # kT free-dim chunks (N per matmul)
    exp_t = mybir.ActivationFunctionType.Exp
    op_add = mybir.AluOpType.add
    op_mult = mybir.AluOpType.mult
    op_max = mybir.AluOpType.max

    with tile.TileContext(nc) as tc:
        with (
            tc.tile_pool(name="persist", bufs=1) as persist,
            tc.tile_pool(name="xtp", bufs=1) as xtp,
            tc.tile_pool(name="wpool", bufs=1) as wpool,
            tc.tile_pool(name="rp", bufs=2) as rp,
            tc.tile_pool(name="ep", bufs=3) as ep,
            tc.tile_pool(name="rcp", bufs=3) as rcp,
            tc.tile_pool(name="bcp", bufs=3) as bcp,
            tc.tile_pool(name="outp", bufs=3) as outp,
            tc.tile_pool(name="psum", bufs=1, space="PSUM") as psum,
        ):
            mbt = persist.tile([128, lkt], F32, tag="mbt", name="mbt")
            qTt = [persist.tile([128, L], BF16, tag=f"qT{i}", name=f"qT{i}")
                   for i in range(KT)]
            kTt = [persist.tile([128, lk], BF16, tag=f"kT{i}", name=f"kT{i}")
                   for i in range(KT)]
            vaug = [persist.tile([128, NH, DH + 1], BF16, tag=f"va{i}", name=f"va{i}")
                    for i in range(lkt)]
            attnT = [persist.tile([128, L], BF16, tag=f"aT{i}", name=f"aT{i}")
                     for i in range(KT)]
            ones_s = persist.tile([128, NH], F32, tag="ones", name="ones")
            nc.vector.memset(ones_s[:], 1.0)
            zeros_t = persist.tile([128, L], BF16, tag="zeros", name="zeros")
            nc.vector.memset(zeros_t[:], 0.0)
            bq_sb = persist.tile([128, KT], F32, tag="bq", name="bq_sb")
            bk_sb = persist.tile([128, KT], F32, tag="bk", name="bk_sb")
            woa = persist.tile([128, KT, H], BF16, tag="woa", name="woa")
            woTt = [woa[:, k, :] for k in range(KT)]

            xTa = xtp.tile([128, KT, L], BF16, tag="xTa", name="xTa")
            xKa = xtp.tile([128, KT, lk], BF16, tag="xKa", name="xKa")
            # q/k weights as per-head-pair column blocks so pair p's
            # projections only wait on their own 196KB, not the full WqT/WkT
            wqp = [wpool.tile([128, KT, 128], BF16, tag=f"wqp{p}",
                              name=f"wqp{p}") for p in range(KT)]
            wkp = [wpool.tile([128, KT, 128], BF16, tag=f"wkp{p}",
                              name=f"wkp{p}") for p in range(KT)]
            wva = wpool.tile([128, KT, H], BF16, tag="wva", name="wva")
            xTt = [xTa[:, k, :] for k in range(KT)]
            xKt = [xKa[:, k, :] for k in range(KT)]
            wvt = [wva[:, k, :] for k in range(KT)]

            # --- input DMAs: spread over three queues, in first-use order.
            # sync: compacted keys (k-projection moving operand) + rv;
            # scalar: per-pair q/k weight blocks; gpsimd: xT + v weights +
            # small/late tensors.
            for k in range(KT):
                nc.sync.dma_start(xKa[:, k, :], xTk[:, k, :])
            # only the first two pairs' q/k weight blocks load upfront;
            # later pairs' blocks are issued from emit_qk(p-2) below so
            # they don't displace the front-critical x bytes.
            for p in range(2):
                nc.scalar.dma_start(wkp[p][:], wk[:, :, p * 128:(p + 1) * 128])
                nc.scalar.dma_start(wqp[p][:], wq[:, :, p * 128:(p + 1) * 128])
            # small tensors ride the scalar queue behind the first weight
            # blocks (all needed only by ~15us: exp bias, psum-add biases)
            nc.scalar.dma_start(mbt[:], mb[:])
            nc.scalar.dma_start(bk_sb[:], bkp[:])
            nc.scalar.dma_start(bq_sb[:], bqp[:])
            for k in range(KT):
                nc.gpsimd.dma_start(xTa[:, k, :], xT[:, k, :])
            # v weights ride the sync queue, which is idle once the
            # compacted keys have landed; behind xTa they'd arrive ~4us
            # too late for the v projection.
            for k in range(KT):
                nc.sync.dma_start(wva[:, k, :], wv[:, k, :])
            if has_bo:
                bo_bc = persist.tile([128, H], F32, tag="bo", name="bo_bc")
                bo_ap = bo_d.ap()
                nc.gpsimd.dma_start(
                    out=bo_bc[:],
                    in_=bass.AP(tensor=bo_ap.tensor, offset=0, ap=[[0, 128], [1, H]]),
                )

            # ---- v projection, natural layout [lk, H], augmented tiles ----
            def emit_v(lt):
                rows = rows_of[lt]
                rv_t = rp.tile([128, H], BF16, tag="rv", name="rv_t")
                nc.gpsimd.dma_start(rv_t[0:rows, :],
                                    rv[lt * 128:lt * 128 + rows, :])
                for ch in range(2):
                    ps = psum.tile([128, 512], F32, tag="ps", bufs=2, name="psv")
                    for k in range(KT):
                        nc.tensor.matmul(
                            ps[0:rows, 0:384],
                            xKt[k][:, lt * 128:lt * 128 + rows],
                            wvt[k][:, ch * 384:(ch + 1) * 384],
                            start=(k == 0), stop=(k == KT - 1),
                        )
                    nc.vector.tensor_add(
                        vaug[lt][0:rows, ch * 6:(ch + 1) * 6, 0:DH],
                        ps[0:rows, 0:384].rearrange("p (h d) -> p h d", d=DH),
                        rv_t[0:rows, ch * 384:(ch + 1) * 384].rearrange(
                            "p (h d) -> p h d", d=DH),
                    )
                nc.vector.tensor_copy(vaug[lt][0:rows, :, DH], ones_s[0:rows, :])

            def emit_qk(p, ramp=False):
                """q/k projections for head-pair p (= ho-tile p of each).

                ramp=True (first pairs, before any scores exist): allocate
                the q-side psum chunks from the idle "st" banks so the q
                and k projections accumulate concurrently while their
                input slabs trickle in from HBM, instead of q waiting for
                the k chunks to release the "ps" buffers."""
                if p + 2 < KT:
                    pn = p + 2
                    nc.scalar.dma_start(
                        wkp[pn][:], wk[:, :, pn * 128:(pn + 1) * 128])
                    nc.scalar.dma_start(
                        wqp[pn][:], wq[:, :, pn * 128:(pn + 1) * 128])
                for wt, b_sb, dst, rhs, xsrc, ck in (
                    (wkp, bk_sb, kTt, xKt, xKa, kchunks),
                    (wqp, bq_sb, qTt, xTt, xTa, ((0, 512), (512, 512))),
                ):
                    is_q = dst is qTt
                    nfree = ck[-1][0] + ck[-1][1]
                    r_t = rp.tile([128, L], BF16, tag="r", name="r_t")
                    # r = 0.5*relu(x) on-core: (x * 0.5) max 0
                    nc.vector.scalar_tensor_tensor(
                        r_t[:, 0:nfree], xsrc[:, p, 0:nfree], 0.5,
                        zeros_t[:, 0:nfree], op_mult, op_max)
                    for (o0, on) in ck:
                        if ramp and is_q:
                            pst = psum.tile([128, L], F32, tag="st", bufs=2,
                                            name="st_ps")
                            ps = pst[:, 0:512]
                        else:
                            ps = psum.tile([128, 512], F32, tag="ps", bufs=2,
                                           name="psq")
                        for k in range(KT):
                            nc.tensor.matmul(
                                ps[:, 0:on],
                                wt[p][:, k, :],
                                rhs[k][:, o0:o0 + on],
                                start=(k == 0), stop=(k == KT - 1),
                            )
                        # dst = (psum + bias_per_partition) + r
                        nc.vector.scalar_tensor_tensor(
                            dst[p][:, o0:o0 + on], ps[:, 0:on],
                            b_sb[:, p:p + 1], r_t[:, o0:o0 + on],
                            op_add, op_add)

            def emit_st(p):
                """Scores + exp for head pair p; returns exp tiles.

                hh-major so the exp tiles drain in the same order emit_pv
                consumes them (head hh's PV needs ex[hh, all i]); this lets
                PV start after 5 exps instead of 9."""
                ex = {}
                for hh, off in ((0, 0), (1, 64)):
                    for i in range(lkt):
                        rows = rows_of[i]
                        pss = psum.tile([128, L], F32, tag="st", bufs=2,
                                        name="st_ps")
                        for j in range(2):
                            nc.tensor.matmul(
                                pss[0:rows, j * 512:(j + 1) * 512],
                                kTt[p][off:off + DH, i * 128:i * 128 + rows],
                                qTt[p][off:off + DH, j * 512:(j + 1) * 512],
                                start=True, stop=True,
                            )
                        e = ep.tile([128, L], BF16, tag=f"ex{hh}_{i}",
                                    name=f"ex{hh}_{i}")
                        nc.scalar.activation(
                            e[0:rows, :], pss[0:rows, :], exp_t,
                            bias=mbt[0:rows, i:i + 1], scale=SCALE)
                        ex[hh, i] = e
                return ex

            def emit_pv(p, ex, tail=False):
                """PV + normalization for head pair p -> attnT.

                tail=True (last pairs, once the exp stream has drained and
                the scalar engine is idle): evacuate the PV PSUM tile to
                SBUF with a scalar copy so the bank frees at matmul rate
                instead of being held through the whole normalize chain."""
                for hh, off in ((0, 0), (1, 64)):
                    head = 2 * p + hh
                    for j in range(2):
                        pv = psum.tile([DH + 1, 512], F32, tag="pv",
                                       bufs=2, name="pv_ps")
                        for i in range(lkt):
                            rows = rows_of[i]
                            nc.tensor.matmul(
                                pv[:],
                                vaug[i][0:rows, head, :],
                                ex[hh, i][0:rows, j * 512:(j + 1) * 512],
                                start=(i == 0), stop=(i == lkt - 1),
                            )
                        if tail:
                            src = rcp.tile([DH + 1, 512], F32, tag="pvs",
                                           name="pvs_t")
                            nc.scalar.copy(src[:], pv[:])
                        else:
                            src = pv
                        # normalize: denom row -> partition-0 SBUF tile (the
                        # custom-DVE reciprocal needs a partition-0-based
                        # SBUF input), reciprocal, broadcast, multiply.
                        dn = rcp.tile([1, 512], F32, tag="dn", name="dn_t")
                        nc.vector.tensor_copy(dn[:], src[DH:DH + 1, :])
                        rc = rcp.tile([1, 512], F32, tag="rc", name="rc_t")
                        nc.vector.reciprocal_approx_fast(out=rc[:], in_=dn[:])
                        bc = bcp.tile([DH, 512], F32, tag="bc", name="bc_t")
                        nc.gpsimd.partition_broadcast(bc[:], rc[:])
                        nc.vector.tensor_mul(
                            attnT[p][off:off + DH, j * 512:(j + 1) * 512],
                            src[0:DH, :], bc[:])

            # software pipeline: scores/exp run one-to-two pairs ahead of
            # PV so the scalar engine's exp stream hides under PE matmuls;
            # st5 is pulled before pv3/pv4 so exp(5) has PE work to hide
            # under at the tail.  The v projection runs after st1 so its
            # input DMAs don't compete with the q/k path for HBM bandwidth
            # during the ramp.
            emit_qk(0, ramp=True)
            exs = {0: emit_st(0)}
            emit_qk(1, ramp=True)
            exs[1] = emit_st(1)
            for lt in range(lkt):
                emit_v(lt)
            # Wo load: issued here (gpsimd reaches it early) but only
            # needed by the out-projection, so it streams in background.
            nc.gpsimd.dma_start(woa[:], wo[:])
            emit_pv(0, exs.pop(0))
            emit_qk(2)
            exs[2] = emit_st(2)
            emit_pv(1, exs.pop(1))
            emit_qk(3)
            exs[3] = emit_st(3)
            emit_pv(2, exs.pop(2))
            emit_qk(4)
            exs[4] = emit_st(4)
            emit_qk(5)
            exs[5] = emit_st(5)
            emit_pv(3, exs.pop(3))
            emit_pv(4, exs.pop(4))
            emit_pv(5, exs.pop(5))

            # ---------------- output projection ----------------
            # the first two row-tiles' accumulators live on the "st" banks
            # (idle once the exp stream drains) so four chunks can prefill
            # their k<=4 partial sums under the last PV pairs instead of two
            for lt in range(LQT):
                so = outp.tile([128, H], F32, tag="so", name="so_t")
                for (o0, on) in ((0, 512), (512, 256)):
                    if lt < 2:
                        pst = psum.tile([128, L], F32, tag="st", bufs=2,
                                        name="st_ps")
                        ps = pst[:, 0:512]
                    else:
                        ps = psum.tile([128, 512], F32, tag="ps", bufs=2,
                                       name="pc")
                    for k in range(KT):
                        nc.tensor.matmul(
                            ps[:, 0:on],
                            attnT[k][:, lt * 128:(lt + 1) * 128],
                            woTt[k][:, o0:o0 + on],
                            start=(k == 0), stop=(k == KT - 1),
                        )
                    if has_bo:
                        nc.vector.tensor_add(
                            so[:, o0:o0 + on], ps[:, 0:on], bo_bc[:, o0:o0 + on])
                    else:
                        nc.vector.tensor_copy(so[:, o0:o0 + on], ps[:, 0:on])
                # alternate output DMAs over the two HW queues so the
                # final drain is ~2x faster; the last tile splits across
                # both queues since nothing else is left to overlap it
                if lt == LQT - 1:
                    nc.sync.dma_start(
                        out_d[lt * 128:(lt + 1) * 128, 0:384], so[:, 0:384])
                    nc.scalar.dma_start(
                        out_d[lt * 128:(lt + 1) * 128, 384:H], so[:, 384:H])
                else:
                    eng = nc.sync if lt % 2 == 0 else nc.scalar
                    eng.dma_start(
                        out_d[lt * 128:(lt + 1) * 128, :], so[:])

    nc.compile()
    return nc


def kernel(hidden_states, attention_mask, Wq, bq, Wk, bk, Wv, bv, Wo, bo):
    global LAST_EXEC_NS, LAST_RESULTS
    x = np.ascontiguousarray(np.asarray(hidden_states, dtype=np.float32))
    mask = np.asarray(attention_mask).astype(bool).reshape(B, L)
    bq = np.asarray(bq, dtype=np.float32)
    bk = np.asarray(bk, dtype=np.float32)
    bv = np.asarray(bv, dtype=np.float32)
    bo = np.asarray(bo, dtype=np.float32)
    has_bo = bool(np.any(bo))

    keep = [np.nonzero(~mask[b])[0] for b in range(B)]
    n_max = max(max(len(k) for k in keep), 64)
    lk = max(128, -(-n_max // 128) * 128)   # padded key count, multiple of 128

    key = (lk, n_max, has_bo)
    if key not in _CACHE:
        _CACHE[key] = _build(lk, n_max, has_bo)
    nc = _CACHE[key]

    bf = ml_dtypes.bfloat16

    def pk(a):
        """[H, X] -> [128, KT, X] (row-tile packing)."""
        return np.ascontiguousarray(
            a.reshape(KT, 128, a.shape[1]).swapaxes(0, 1))

    def pb(b_):
        """[H] -> [128, KT] per-slab bias packing."""
        return np.ascontiguousarray(b_.reshape(KT, 128).T)

    wqT = pk(np.asarray(Wq, dtype=np.float32).T.astype(bf))
    wkT = pk(np.asarray(Wk, dtype=np.float32).T.astype(bf))
    wvT = pk(np.asarray(Wv, dtype=np.float32).T.astype(bf))
    woT = pk(np.asarray(Wo, dtype=np.float32).T.astype(bf))
    bqpk = pb(bq)
    bkpk = pb(bk)

    in_maps = []
    for b in range(B):
        xb = x[b]                               # [L, H]
        rb = 0.5 * np.maximum(xb, 0.0)          # [L, H]
        idx = keep[b]
        n = len(idx)
        xk = np.zeros((lk, H), np.float32)      # compacted+padded key rows
        xk[:n] = xb[idx]
        rvb = np.zeros((lk, H), np.float32)
        rvb[:n] = rb[idx] + bv[None, :]
        mbias = np.full((lk,), NEG, np.float32)
        mbias[:n] = 0.0
        in_maps.append({
            "xT": pk(xb.T.astype(bf)),
            "xTk": pk(xk.T.astype(bf)),
            "rv": rvb.astype(bf),
            "wqT": wqT, "wkT": wkT, "wvT": wvT, "woT": woT,
            "bqp": bqpk, "bkp": bkpk,
            "maskb": np.ascontiguousarray(mbias.reshape(lk // 128, 128).T),
            **({"bo": bo} if has_bo else {}),
        })

    trace = bool(os.environ.get("BASS_KERNEL_TRACE"))
    res = run_bass_kernel_spmd(nc, in_maps, list(range(B)), trace=trace)
    LAST_EXEC_NS = res.exec_time_ns
    LAST_RESULTS = res
    return np.stack([res.results[b]["out"] for b in range(B)], axis=0)


# revision 43
# speedup vs baseline: 1.0340x; 1.0138x over previous
"""BertSelfAttention (with value-bypass relu-add) on 8 Trainium2 NeuronCores.

Strategy: data-parallel over batch B=8 -> one batch element per core, no
collectives. Per core, attention is computed in a transposed-softmax layout:

  qT, kT = (x @ W.T).T + b + r.T      [H, L] (heads are 64-row slices)
  v      = x @ Wv.T + r               [Lk, H], augmented with a ones column
  S.T    = kT_head.T-matmul           [lk, lq]  (keys on partitions)
  E      = exp(S.T * 1/8 + maskbias)  (mask folded into the activation bias;
                                       exp(-1e9) == 0 kills masked keys)
  PV     = [v_head | 1].T @ E         -> rows 0..63 unnormalized attn.T,
                                         row 64 = softmax denominator (free)
  attnT  = PV[0:64] * bcast(1/PV[64]) (approx-recip + gpsimd partition bcast)
  out    = attnT.T-matmul with Wo.T + bo

Masked keys are compacted away on the host (gather unmasked key rows, pad to
a multiple of 128; padded keys get x=0 and a -1e9 bias so exp()==0 exactly).

The relu bypass r = 0.5*relu(x) is computed ON-CORE from the already-loaded
x tiles (relu(0.5x) == 0.5relu(x)) as a DVE (vector) op, so no rqT/rkT DMA.
Per-partition q/k biases are folded into the psum+r add via
scalar_tensor_tensor.

Input DMAs are split per k-slab / per head-pair weight block and spread
across the sync/scalar HW DGE queues plus the gpsimd SW queue, ordered so
the k-projection's operands land first and later pairs' weights stream
just-in-time from inside the pipeline; Wo loads in background (needed only
at the out-projection); output DMAs alternate between the two HW queues.

Pipeline order: qk0 st0 qk1 st1 v* | pv0 qk2 st2 | pv1 qk3 st3 | pv2 qk4
st4 | qk5 st5 pv3 pv4 pv5 | out-proj — scores/exp run 1-2 pairs ahead of PV
so the scalar engine's exp stream and the PV normalize chains hide under PE
matmuls; the Tile scheduler additionally prefills out-projection
accumulation (k<=4) into the late-PV bubbles.
"""

import os
import sys

for _p in ("/opt/trn_rl_repo", "/root/.axon_site/_ro/trn_rl_repo"):
    if os.path.isdir(_p) and _p not in sys.path:
        sys.path.insert(0, _p)

import ml_dtypes
import numpy as np

import concourse.bacc as bacc
import concourse.bass as bass
import concourse.mybir as mybir
import concourse.tile as tile
from concourse.bass_utils import run_bass_kernel_spmd

B, L, H = 8, 1024, 768
NH, DH = 12, 64
SCALE = 1.0 / 8.0
NEG = -1e9
KT = H // 128            # 6 contraction tiles over hidden dim
LQT = L // 128           # 8 query row-tiles
F32 = mybir.dt.float32
BF16 = mybir.dt.bfloat16

LAST_EXEC_NS = None
LAST_RESULTS = None
_CACHE = {}


def _chunks(total, maxc):
    """Split `total` into nearly-equal chunks of at most `maxc`, multiples of 64."""
    n = -(-total // maxc)
    base = total // n
    base -= base % 64
    sizes = [base] * n
    sizes[-1] = total - base * (n - 1)
    out, off = [], 0
    for s in sizes:
        out.append((off, s))
        off += s
    return out


def _build(lk, nmax, has_bo):
    """Build + compile the 8-core SPMD program; lk = padded key count
    (tile allocation), nmax = max real key count (compute bound)."""
    lkt = lk // 128          # key row-tiles
    rows_of = [min(128, nmax - 128 * i) for i in range(lkt)]
    nc = bacc.Bacc("TRN2", target_bir_lowering=False, debug=False, num_devices=B)

    xT = nc.dram_tensor("xT", [128, KT, L], BF16, kind="ExternalInput")
    xTk = nc.dram_tensor("xTk", [128, KT, lk], BF16, kind="ExternalInput")
    rv = nc.dram_tensor("rv", [lk, H], BF16, kind="ExternalInput")
    wq = nc.dram_tensor("wqT", [128, KT, H], BF16, kind="ExternalInput")
    wk = nc.dram_tensor("wkT", [128, KT, H], BF16, kind="ExternalInput")
    wv = nc.dram_tensor("wvT", [128, KT, H], BF16, kind="ExternalInput")
    wo = nc.dram_tensor("woT", [128, KT, H], BF16, kind="ExternalInput")
    mb = nc.dram_tensor("maskb", [128, lkt], F32, kind="ExternalInput")
    bqp = nc.dram_tensor("bqp", [128, KT], F32, kind="ExternalInput")
    bkp = nc.dram_tensor("bkp", [128, KT], F32, kind="ExternalInput")
    out_d = nc.dram_tensor("out", [L, H], F32, kind="ExternalOutput")
    bo_d = nc.dram_tensor("bo", [H], F32, kind="ExternalInput") if has_bo else None

    kchunks = _chunks(nmax, 512)     # kT free-dim chunks (N per matmul)
    exp_t = mybir.ActivationFunctionType.Exp
    op_add = mybir.AluOpType.add
    op_mult = mybir.AluOpType.mult
    op_max = mybir.AluOpType.max

    with tile.TileContext(nc) as tc:
        with (
            tc.tile_pool(name="persist", bufs=1) as persist,
            tc.tile_pool(name="xtp", bufs=1) as xtp,
            tc.tile_pool(name="wpool", bufs=1) as wpool,
            tc.tile_pool(name="rp", bufs=2) as rp,
            tc.tile_pool(name="ep", bufs=3) as ep,
            tc.tile_pool(name="rcp", bufs=3) as rcp,
            tc.tile_pool(name="bcp", bufs=3) as bcp,
            tc.tile_pool(name="outp", bufs=3) as outp,
            tc.tile_pool(name="psum", bufs=1, space="PSUM") as psum,
        ):
            mbt = persist.tile([128, lkt], F32, tag="mbt", name="mbt")
            qTt = [persist.tile([128, L], BF16, tag=f"qT{i}", name=f"qT{i}")
                   for i in range(KT)]
            kTt = [persist.tile([128, lk], BF16, tag=f"kT{i}", name=f"kT{i}")
                   for i in range(KT)]
            vaug = [persist.tile([128, NH, DH + 1], BF16, tag=f"va{i}", name=f"va{i}")
                    for i in range(lkt)]
            attnT = [persist.tile([128, L], BF16, tag=f"aT{i}", name=f"aT{i}")
                     for i in range(KT)]
            ones_s = persist.tile([128, NH], F32, tag="ones", name="ones")
            nc.vector.memset(ones_s[:], 1.0)
            zeros_t = persist.tile([128, L], BF16, tag="zeros", name="zeros")
            nc.vector.memset(zeros_t[:], 0.0)
            bq_sb = persist.tile([128, KT], F32, tag="bq", name="bq_sb")
            bk_sb = persist.tile([128, KT], F32, tag="bk", name="bk_sb")
            woa = persist.tile([128, KT, H], BF16, tag="woa", name="woa")
            woTt = [woa[:, k, :] for k in range(KT)]

            xTa = xtp.tile([128, KT, L], BF16, tag="xTa", name="xTa")
            xKa = xtp.tile([128, KT, lk], BF16, tag="xKa", name="xKa")
            # q/k weights as per-head-pair column blocks so pair p's
            # projections only wait on their own 196KB, not the full WqT/WkT
            wqp = [wpool.tile([128, KT, 128], BF16, tag=f"wqp{p}",
                              name=f"wqp{p}") for p in range(KT)]
            wkp = [wpool.tile([128, KT, 128], BF16, tag=f"wkp{p}",
                              name=f"wkp{p}") for p in range(KT)]
            wva = wpool.tile([128, KT, H], BF16, tag="wva", name="wva")
            xTt = [xTa[:, k, :] for k in range(KT)]
            xKt = [xKa[:, k, :] for k in range(KT)]
            wvt = [wva[:, k, :] for k in range(KT)]

            # --- input DMAs: spread over three queues, in first-use order.
            # sync: compacted keys (k-projection moving operand) + rv;
            # scalar: per-pair q/k weight blocks; gpsimd: xT + v weights +
            # small/late tensors.
            for k in range(KT):
                nc.sync.dma_start(xKa[:, k, :], xTk[:, k, :])
            # only the first two pairs' q/k weight blocks load upfront;
            # later pairs' blocks are issued from emit_qk(p-2) below so
            # they don't displace the front-critical x bytes.
            for p in range(2):
                nc.scalar.dma_start(wkp[p][:], wk[:, :, p * 128:(p + 1) * 128])
                nc.scalar.dma_start(wqp[p][:], wq[:, :, p * 128:(p + 1) * 128])
            # small tensors ride the scalar queue behind the first weight
            # blocks (all needed only by ~15us: exp bias, psum-add biases)
            nc.scalar.dma_start(mbt[:], mb[:])
            nc.scalar.dma_start(bk_sb[:], bkp[:])
            nc.scalar.dma_start(bq_sb[:], bqp[:])
            for k in range(KT):
                nc.gpsimd.dma_start(xTa[:, k, :], xT[:, k, :])
            # v weights ride the sync queue, which is idle once the
            # compacted keys have landed; behind xTa they'd arrive ~4us
            # too late for the v projection.
            for k in range(KT):
                nc.sync.dma_start(wva[:, k, :], wv[:, k, :])
            if has_bo:
                bo_bc = persist.tile([128, H], F32, tag="bo", name="bo_bc")
                bo_ap = bo_d.ap()
                nc.gpsimd.dma_start(
                    out=bo_bc[:],
                    in_=bass.AP(tensor=bo_ap.tensor, offset=0, ap=[[0, 128], [1, H]]),
                )

            # ---- v projection, natural layout [lk, H], augmented tiles ----
            def emit_v(lt):
                rows = rows_of[lt]
                rv_t = rp.tile([128, H], BF16, tag="rv", name="rv_t")
                nc.gpsimd.dma_start(rv_t[0:rows, :],
                                    rv[lt * 128:lt * 128 + rows, :])
                for ch in range(2):
                    ps = psum.tile([128, 512], F32, tag="ps", bufs=2, name="psv")
                    for k in range(KT):
                        nc.tensor.matmul(
                            ps[0:rows, 0:384],
                            xKt[k][:, lt * 128:lt * 128 + rows],
                            wvt[k][:, ch * 384:(ch + 1) * 384],
                            start=(k == 0), stop=(k == KT - 1),
                        )
                    nc.vector.tensor_add(
                        vaug[lt][0:rows, ch * 6:(ch + 1) * 6, 0:DH],
                        ps[0:rows, 0:384].rearrange("p (h d) -> p h d", d=DH),
                        rv_t[0:rows, ch * 384:(ch + 1) * 384].rearrange(
                            "p (h d) -> p h d", d=DH),
                    )
                nc.vector.tensor_copy(vaug[lt][0:rows, :, DH], ones_s[0:rows, :])

            def emit_qk(p, ramp=False):
                """q/k projections for head-pair p (= ho-tile p of each).

                ramp=True (first pairs, before any scores exist): allocate
                the q-side psum chunks from the idle "st" banks so the q
                and k projections accumulate concurrently while their
                input slabs trickle in from HBM, instead of q waiting for
                the k chunks to release the "ps" buffers."""
                if p + 2 < KT:
                    # deferred weight blocks ride sync (idle mid-kernel) so
                    # their issue cost never delays the scalar exp stream
                    pn = p + 2
                    nc.sync.dma_start(
                        wkp[pn][:], wk[:, :, pn * 128:(pn + 1) * 128])
                    nc.sync.dma_start(
                        wqp[pn][:], wq[:, :, pn * 128:(pn + 1) * 128])
                for wt, b_sb, dst, rhs, xsrc, ck in (
                    (wkp, bk_sb, kTt, xKt, xKa, kchunks),
                    (wqp, bq_sb, qTt, xTt, xTa, ((0, 512), (512, 512))),
                ):
                    is_q = dst is qTt
                    nfree = ck[-1][0] + ck[-1][1]
                    r_t = rp.tile([128, L], BF16, tag="r", name="r_t")
                    # r = 0.5*relu(x) on-core: (x * 0.5) max 0
                    nc.vector.scalar_tensor_tensor(
                        r_t[:, 0:nfree], xsrc[:, p, 0:nfree], 0.5,
                        zeros_t[:, 0:nfree], op_mult, op_max)
                    for (o0, on) in ck:
                        if ramp and is_q:
                            pst = psum.tile([128, L], F32, tag="st", bufs=2,
                                            name="st_ps")
                            ps = pst[:, 0:512]
                        else:
                            ps = psum.tile([128, 512], F32, tag="ps", bufs=2,
                                           name="psq")
                        for k in range(KT):
                            nc.tensor.matmul(
                                ps[:, 0:on],
                                wt[p][:, k, :],
                                rhs[k][:, o0:o0 + on],
                                start=(k == 0), stop=(k == KT - 1),
                            )
                        # dst = (psum + bias_per_partition) + r
                        nc.vector.scalar_tensor_tensor(
                            dst[p][:, o0:o0 + on], ps[:, 0:on],
                            b_sb[:, p:p + 1], r_t[:, o0:o0 + on],
                            op_add, op_add)

            def emit_st(p):
                """Scores + exp for head pair p; returns exp tiles.

                hh-major so the exp tiles drain in the same order emit_pv
                consumes them (head hh's PV needs ex[hh, all i]); this lets
                PV start after 5 exps instead of 9."""
                ex = {}
                for hh, off in ((0, 0), (1, 64)):
                    for i in range(lkt):
                        rows = rows_of[i]
                        pss = psum.tile([128, L], F32, tag="st", bufs=2,
                                        name="st_ps")
                        for j in range(2):
                            nc.tensor.matmul(
                                pss[0:rows, j * 512:(j + 1) * 512],
                                kTt[p][off:off + DH, i * 128:i * 128 + rows],
                                qTt[p][off:off + DH, j * 512:(j + 1) * 512],
                                start=True, stop=True,
                            )
                        e = ep.tile([128, L], BF16, tag=f"ex{hh}_{i}",
                                    name=f"ex{hh}_{i}")
                        nc.scalar.activation(
                            e[0:rows, :], pss[0:rows, :], exp_t,
                            bias=mbt[0:rows, i:i + 1], scale=SCALE)
                        ex[hh, i] = e
                return ex

            def emit_pv(p, ex, tail=False):
                """PV + normalization for head pair p -> attnT.

                tail=True (last pairs, once the exp stream has drained and
                the scalar engine is idle): evacuate the PV PSUM tile to
                SBUF with a scalar copy so the bank frees at matmul rate
                instead of being held through the whole normalize chain."""
                for hh, off in ((0, 0), (1, 64)):
                    head = 2 * p + hh
                    for j in range(2):
                        pv = psum.tile([DH + 1, 512], F32, tag="pv",
                                       bufs=2, name="pv_ps")
                        for i in range(lkt):
                            rows = rows_of[i]
                            nc.tensor.matmul(
                                pv[:],
                                vaug[i][0:rows, head, :],
                                ex[hh, i][0:rows, j * 512:(j + 1) * 512],
                                start=(i == 0), stop=(i == lkt - 1),
                            )
                        if tail:
                            src = rcp.tile([DH + 1, 512], F32, tag="pvs",
                                           name="pvs_t")
                            nc.scalar.copy(src[:], pv[:])
                        else:
                            src = pv
                        # normalize: denom row -> partition-0 SBUF tile (the
                        # custom-DVE reciprocal needs a partition-0-based
                        # SBUF input), reciprocal, broadcast, multiply.
                        dn = rcp.tile([1, 512], F32, tag="dn", name="dn_t")
                        nc.vector.tensor_copy(dn[:], src[DH:DH + 1, :])
                        rc = rcp.tile([1, 512], F32, tag="rc", name="rc_t")
                        nc.vector.reciprocal_approx_fast(out=rc[:], in_=dn[:])
                        bc = bcp.tile([DH, 512], F32, tag="bc", name="bc_t")
                        nc.gpsimd.partition_broadcast(bc[:], rc[:])
                        nc.vector.tensor_mul(
                            attnT[p][off:off + DH, j * 512:(j + 1) * 512],
                            src[0:DH, :], bc[:])

            # software pipeline: scores/exp run one-to-two pairs ahead of
            # PV so the scalar engine's exp stream hides under PE matmuls;
            # st5 is pulled before pv3/pv4 so exp(5) has PE work to hide
            # under at the tail.  The v projection runs after st1 so its
            # input DMAs don't compete with the q/k path for HBM bandwidth
            # during the ramp.
            emit_qk(0, ramp=True)
            exs = {0: emit_st(0)}
            emit_qk(1, ramp=True)
            exs[1] = emit_st(1)
            for lt in range(lkt):
                emit_v(lt)
            # Wo load: issued here (gpsimd reaches it early) but only
            # needed by the out-projection, so it streams in background.
            nc.gpsimd.dma_start(woa[:], wo[:])
            emit_pv(0, exs.pop(0))
            emit_qk(2)
            exs[2] = emit_st(2)
            emit_pv(1, exs.pop(1))
            emit_qk(3)
            exs[3] = emit_st(3)
            emit_pv(2, exs.pop(2))
            emit_qk(4)
            exs[4] = emit_st(4)
            emit_qk(5)
            exs[5] = emit_st(5)
            emit_pv(3, exs.pop(3))
            emit_pv(4, exs.pop(4))
            emit_pv(5, exs.pop(5))

            # ---------------- output projection ----------------
            # the first two row-tiles' accumulators live on the "st" banks
            # (idle once the exp stream drains) so four chunks can prefill
            # their k<=4 partial sums under the last PV pairs instead of two
            for lt in range(LQT):
                so = outp.tile([128, H], F32, tag="so", name="so_t")
                for (o0, on) in ((0, 512), (512, 256)):
                    if lt < 2:
                        pst = psum.tile([128, L], F32, tag="st", bufs=2,
                                        name="st_ps")
                        ps = pst[:, 0:512]
                    else:
                        ps = psum.tile([128, 512], F32, tag="ps", bufs=2,
                                       name="pc")
                    for k in range(KT):
                        nc.tensor.matmul(
                            ps[:, 0:on],
                            attnT[k][:, lt * 128:(lt + 1) * 128],
                            woTt[k][:, o0:o0 + on],
                            start=(k == 0), stop=(k == KT - 1),
                        )
                    if has_bo:
                        nc.vector.tensor_add(
                            so[:, o0:o0 + on], ps[:, 0:on], bo_bc[:, o0:o0 + on])
                    else:
                        nc.vector.tensor_copy(so[:, o0:o0 + on], ps[:, 0:on])
                # alternate output DMAs over the two HW queues so the
                # final drain is ~2x faster; the last tile splits across
                # both queues since nothing else is left to overlap it
                if lt == LQT - 1:
                    nc.sync.dma_start(
                        out_d[lt * 128:(lt + 1) * 128, 0:384], so[:, 0:384])
                    nc.scalar.dma_start(
                        out_d[lt * 128:(lt + 1) * 128, 384:H], so[:, 384:H])
                else:
                    eng = nc.sync if lt % 2 == 0 else nc.scalar
                    eng.dma_start(
                        out_d[lt * 128:(lt + 1) * 128, :], so[:])

    nc.compile()
    return nc


def kernel(hidden_states, attention_mask, Wq, bq, Wk, bk, Wv, bv, Wo, bo):
    global LAST_EXEC_NS, LAST_RESULTS
    x = np.ascontiguousarray(np.asarray(hidden_states, dtype=np.float32))
    mask = np.asarray(attention_mask).astype(bool).reshape(B, L)
    bq = np.asarray(bq, dtype=np.float32)
    bk = np.asarray(bk, dtype=np.float32)
    bv = np.asarray(bv, dtype=np.float32)
    bo = np.asarray(bo, dtype=np.float32)
    has_bo = bool(np.any(bo))

    keep = [np.nonzero(~mask[b])[0] for b in range(B)]
    n_max = max(max(len(k) for k in keep), 64)
    lk = max(128, -(-n_max // 128) * 128)   # padded key count, multiple of 128

    key = (lk, n_max, has_bo)
    if key not in _CACHE:
        _CACHE[key] = _build(lk, n_max, has_bo)
    nc = _CACHE[key]

    bf = ml_dtypes.bfloat16

    def pk(a):
        """[H, X] -> [128, KT, X] (row-tile packing)."""
        return np.ascontiguousarray(
            a.reshape(KT, 128, a.shape[1]).swapaxes(0, 1))

    def pb(b_):
        """[H] -> [128, KT] per-slab bias packing."""
        return np.ascontiguousarray(b_.reshape(KT, 128).T)

    wqT = pk(np.asarray(Wq, dtype=np.float32).T.astype(bf))
    wkT = pk(np.asarray(Wk, dtype=np.float32).T.astype(bf))
    wvT = pk(np.asarray(Wv, dtype=np.float32).T.astype(bf))
    woT = pk(np.asarray(Wo, dtype=np.float32).T.astype(bf))
    bqpk = pb(bq)
    bkpk = pb(bk)

    in_maps = []
    for b in range(B):
        xb = x[b]                               # [L, H]
        rb = 0.5 * np.maximum(xb, 0.0)          # [L, H]
        idx = keep[b]
        n = len(idx)
        xk = np.zeros((lk, H), np.float32)      # compacted+padded key rows
        xk[:n] = xb[idx]
        rvb = np.zeros((lk, H), np.float32)
        rvb[:n] = rb[idx] + bv[None, :]
        mbias = np.full((lk,), NEG, np.float32)
        mbias[:n] = 0.0
        in_maps.append({
            "xT": pk(xb.T.astype(bf)),
            "xTk": pk(xk.T.astype(bf)),
            "rv": rvb.astype(bf),
            "wqT": wqT, "wkT": wkT, "wvT": wvT, "woT": woT,
            "bqp": bqpk, "bkp": bkpk,
            "maskb": np.ascontiguousarray(mbias.reshape(lk // 128, 128).T),
            **({"bo": bo} if has_bo else {}),
        })

    trace = bool(os.environ.get("BASS_KERNEL_TRACE"))
    res = run_bass_kernel_spmd(nc, in_maps, list(range(B)), trace=trace)
    LAST_EXEC_NS = res.exec_time_ns
    LAST_RESULTS = res
    return np.stack([res.results[b]["out"] for b in range(B)], axis=0)
